# revision 32
# baseline (speedup 1.0000x reference)
"""Trainium2 Bass kernel for nn_DecoderLayer_43963285242628.

Decoder layer: RMSNorm -> GQA attention (QK-norm + split-half RoPE, causal)
-> residual -> RMSNorm -> MoE (16 experts, group-limited top-4 sigmoid
routing) + shared SwiGLU expert -> residual.

Distribution over 8 NeuronCores:
- Tokens are striped over cores (permuted position c*256+t <-> abs token
  c+8t) so causal-attention work per core is identical and the whole
  program is static (causality handled by per-core data masks).
- Attention + shared expert: token-parallel (each core its 256 tokens).
- MoE FFN: expert-parallel (2 experts/core): AllGather of t, routing
  replicated on-device, token dispatch via matmul prefix-sum compaction +
  dma_scatter_add/dma_gather, bf16 ReduceScatter of expert outputs.
- Precision: attention chain f32r (tf32-grade; routing needs an accurate
  gate), probabilities/AV bf16, expert + shared FFN bf16, residuals fp32.
"""

import numpy as np
import ml_dtypes

import bass_rust
import concourse.bass as bass
import concourse.mybir as mybir
import concourse.tile as tile
from concourse.bass import ts, ds
from concourse.bass_utils import run_bass_kernel_spmd
from concourse.masks import make_identity

F32 = mybir.dt.float32
F32R = mybir.dt.float32r
BF16 = mybir.dt.bfloat16
I16 = mybir.dt.int16
U32 = mybir.dt.uint32
AX = mybir.AxisListType
ALU = bass.mybir.AluOpType
ACTF = mybir.ActivationFunctionType

# ---- problem constants ----
NCORE = 8
L, D = 2048, 2048
H, KVH, HD = 16, 4, 128
E, TOPK, G, TKG = 16, 4, 4, 2
I_FF, SH_I = 1024, 2048
EPS = 1e-5
THETA = 1e6
SCALE = HD ** -0.5
RSF = 2.5

TPC = L // NCORE          # 256 tokens per core
QT = TPC // 128           # 2 q-tiles
DB = D // 128             # 16
CAP = 768                 # per-expert token capacity
CAPB = CAP // 128         # 6
CPAD = 64

PAIRS = [(11, 1), (10, 15), (9, 14), (12, 13), (8, 5), (0, 2), (6, 7), (3, 4)]
PERM = np.array([c + NCORE * t for c in range(NCORE) for t in range(TPC)])
# k-block order in SBUF: even permuted blocks first, then odd
KORD = [2 * i for i in range(8)] + [2 * i + 1 for i in range(8)]

_MAXW = 1
_uid = [0]


def _patched_drain_and_barrier(self, tick_clock, wait_clock):
    # this walrus build rejects >1 sem wait per instruction; spill the
    # kernel-tail drain's waits onto follow-up SP nops
    nc = self.nc
    drain_inst = nc.sync.drain()
    wait_clock.add_sem_waits(
        drain_inst.ins, tile.ScopedClock({None: tick_clock.global_clock})
    )
    si = drain_inst.ins.sync_info
    waits = list(si.on_wait) if si is not None else []
    if len(waits) > _MAXW:
        si.on_wait = waits[:_MAXW]
        drain_inst.ins.sync_info = si
        for w in waits[_MAXW:]:
            nop = nc.sync.nop(hint="drain_wait_spill", nofuse=True)
            nop.ins.sync_info = bass_rust.SyncInfo(on_wait=[w], on_update=[])
    nc.all_engine_barrier()
    assert self.sems is not None
    popped = nc._tile_sem_poison_stack.pop()
    assert popped is self._sem_poison
    nc.clear_and_free_semaphores(list(self.sems.allocated().values()))
    nc.all_engine_barrier()


tile.TileContext._drain_and_barrier = _patched_drain_and_barrier


def _fixup_multi_waits(nc):
    """Split multi-wait instructions: extras go on inserted same-engine NoOps."""
    n_split = 0
    for fn in nc.m.functions:
        for bb in fn.blocks:
            il = bb.instructions
            i = 0
            while i < len(il):
                ins = il[i]
                si = ins.sync_info
                if si is None:
                    i += 1
                    continue
                waits = list(si.on_wait)
                if len(waits) <= 1:
                    i += 1
                    continue
                si.on_wait = waits[-1:]
                ins.sync_info = si
                for w in waits[:-1]:
                    _uid[0] += 1
                    nop = mybir.InstNoOp(name=f"I-waitspill-{_uid[0]}", ins=[], outs=[])
                    nop.engine = ins.engine
                    nop.sync_info = bass_rust.SyncInfo(on_wait=[w], on_update=[])
                    il.insert(i, nop)
                    i += 1
                n_split += 1
                i += 1
    return n_split


def build(debug=False):
    nc = bass.Bass()
    dp = nc.declare_dram_parameter

    # ---------------- per-core inputs ----------------
    xT32 = dp("xT32", [D, TPC], F32, isOutput=False)
    wqT = dp("wqT", [D, H * HD], BF16, isOutput=False)
    wkT = dp("wkT", [D, KVH * HD], BF16, isOutput=False)
    wvT = dp("wvT", [D, KVH * HD], BF16, isOutput=False)
    woT = dp("woT", [H * HD, D], BF16, isOutput=False)
    gwT = dp("gwT", [D, E], F32R, isOutput=False)
    gbias = dp("gbias", [1, E], F32, isOutput=False)
    ln1pd = dp("ln1pd", [128, DB], F32, isOutput=False)
    ln2pd = dp("ln2pd", [128, DB], F32, isOutput=False)
    qnw = dp("qnw", [1, HD], F32, isOutput=False)
    knw = dp("knw", [1, HD], F32, isOutput=False)
    cosq = dp("cosq", [TPC, HD // 2], F32, isOutput=False)
    sinq = dp("sinq", [TPC, HD // 2], F32, isOutput=False)
    maskp = dp("maskp", [4, 128, 256], BF16, isOutput=False)   # paired causal masks
    esel0 = dp("esel0", [1, CPAD], F32, isOutput=False)
    esel1 = dp("esel1", [1, CPAD], F32, isOutput=False)
    wg_p = dp("wg_p", [2, D, I_FF], BF16, isOutput=False)
    wu_p = dp("wu_p", [2, D, I_FF], BF16, isOutput=False)
    wd_p = dp("wd_p", [2, I_FF, D], BF16, isOutput=False)
    shgT = dp("shgT", [D, SH_I], BF16, isOutput=False)
    shuT = dp("shuT", [D, SH_I], BF16, isOutput=False)
    shdT = dp("shdT", [SH_I, D], BF16, isOutput=False)

    # ---------------- outputs ----------------
    out_sh = dp("out_sh", [TPC, D], F32, isOutput=True)
    if debug:
        dbg_h1T = dp("dbg_h1T", [D, TPC], F32, isOutput=True)
        dbg_t = dp("dbg_t", [TPC, D], BF16, isOutput=True)
        dbg_C = dp("dbg_C", [L, E], F32, isOutput=True)
        dbg_q = dp("dbg_q", [TPC, H * HD], F32, isOutput=True)
        dbg_k = dp("dbg_k", [TPC, KVH * HD], F32, isOutput=True)
        dbg_o = dp("dbg_o", [TPC, H * HD], F32, isOutput=True)
        dbg_y = dp("dbg_y", [TPC, D], BF16, isOutput=True)

    with tile.TileContext(nc) as tc:
        with (
            tc.tile_pool(name="const", bufs=1) as cpool,
            tc.tile_pool(name="big", bufs=1) as big,
            tc.tile_pool(name="wload", bufs=2) as wload,
            tc.tile_pool(name="work", bufs=2) as work,
            tc.tile_pool(name="route", bufs=2) as route,
            tc.tile_pool(name="ps", bufs=2, space="PSUM") as ps,
            tc.tile_pool(name="ps_acc", bufs=1, space="PSUM") as ps_acc,
            tc.tile_pool(name="dram", bufs=1, space="DRAM") as dram,
        ):
            # ======== constants ========
            ident_f = cpool.tile([128, 128], F32)
            make_identity(nc, ident_f[:])
            ident_b = cpool.tile([128, 128], BF16)
            make_identity(nc, ident_b[:])
            ones_f = cpool.tile([128, 128], F32)
            nc.vector.memset(ones_f[:], 1.0)
            ones1 = cpool.tile([1, 128], F32)
            nc.vector.memset(ones1[:], 1.0)

            def bcast_row(row_ap, width, dst):
                pb = ps.tile([128, 512], F32, tag="ps512", name="pb")
                nc.tensor.matmul(pb[:, :width], ones1[:], row_ap, start=True, stop=True)
                nc.vector.tensor_copy(dst[:], pb[:, :width])

            ln1_sb = cpool.tile([128, DB], F32)
            nc.sync.dma_start(ln1_sb[:], ln1pd[:])
            ln2_sb = cpool.tile([128, DB], F32)
            nc.sync.dma_start(ln2_sb[:], ln2pd[:])
            qnw_sb = cpool.tile([128, HD], F32)
            qnw_row = cpool.tile([1, HD], F32)
            nc.sync.dma_start(qnw_row[:], qnw[:])
            bcast_row(qnw_row[:], HD, qnw_sb)
            knw_sb = cpool.tile([128, HD], F32)
            knw_row = cpool.tile([1, HD], F32)
            nc.sync.dma_start(knw_row[:], knw[:])
            bcast_row(knw_row[:], HD, knw_sb)
            cos_sb = cpool.tile([128, QT, HD // 2], F32)
            nc.sync.dma_start(cos_sb[:], cosq.rearrange("(q p) f -> p q f", p=128))
            sin_sb = cpool.tile([128, QT, HD // 2], F32)
            nc.sync.dma_start(sin_sb[:], sinq.rearrange("(q p) f -> p q f", p=128))
            mask_sb = cpool.tile([128, 4, 256], BF16)
            nc.sync.dma_start(mask_sb[:], maskp.rearrange("i p j -> p i j"))
            gb_row = cpool.tile([1, E], F32)
            nc.sync.dma_start(gb_row[:], gbias[:])
            gb_sb = cpool.tile([128, E], F32)
            bcast_row(gb_row[:], E, gb_sb)
            esel_sb = []
            for k, esel_p in enumerate((esel0, esel1)):
                row_ = cpool.tile([1, CPAD], F32, tag=f"eselr{k}", name=f"eselr{k}")
                nc.sync.dma_start(row_[:], esel_p[:])
                t_ = cpool.tile([128, CPAD], F32, tag=f"eselb{k}", name=f"eselb{k}")
                bcast_row(row_[:], CPAD, t_)
                esel_sb.append(t_)
            iota16 = cpool.tile([128, E], F32)
            nc.gpsimd.iota(iota16[:], pattern=[[1, E]], base=0, channel_multiplier=0,
                           allow_small_or_imprecise_dtypes=True)
            gwT_sb = cpool.tile([128, DB, E], F32R)
            nc.sync.dma_start(gwT_sb[:], gwT.rearrange("(b p) e -> p b e", p=128))

            # ======== DRAM internals ========
            ag_kv_in = dram.tile([TPC, 1024], BF16)
            ag_kv_out = dram.tile([L, 1024], BF16)
            ag_t_in = dram.tile([TPC, D], BF16)
            t_full = dram.tile([L, D], BF16)
            ag_s_in = dram.tile([TPC, E], F32)
            sc_full = dram.tile([L, E], F32)
            y_dram = dram.tile([L, D], BF16)
            y_shard = dram.tile([TPC, D], BF16)

            # ======== x^T load + rms -> xn (f32r) ========
            # big-pool slot tags (lifetime chains):
            #   A: xn -> o_nat      B: x_sb(2 gens) -> hm      C: q -> tT_r
            #   K: kT -> tg         V: v_bf -> out_f           Q: qT -> hms
            #   O: oT -> y_sb       H: h1T                     T: tT_bf -> y_sh
            xw = big.tile([128, DB, TPC], F32, tag="C", name="xw")
            nc.sync.dma_start(xw[:], xT32.rearrange("(b p) t -> p b t", p=128))
            ssq_ps = ps_acc.tile([128, TPC], F32, tag="ssq", name="ssq_ps")
            for b in range(DB):
                sqb = work.tile([128, TPC], F32, tag="sqb", name="sqb")
                nc.vector.tensor_tensor(sqb[:], xw[:, b], xw[:, b], ALU.mult)
                nc.tensor.matmul(ssq_ps[:], ones_f[:], sqb[:], start=(b == 0), stop=(b == DB - 1))
            m_t = work.tile([128, TPC], F32, tag="m_rms", name="m_t")
            nc.vector.tensor_scalar(m_t[:], ssq_ps[:], 1.0 / D, EPS, ALU.mult, ALU.add)
            # newton-refined rsqrt (ACT sqrt is loose)
            def rsqrt_refined(m_ap, shape, nm):
                s_ = work.tile(shape, F32, tag="rsq_s", name=f"rs_{nm}")
                nc.scalar.activation(s_[:], m_ap, ACTF.Sqrt)
                r_ = work.tile(shape, F32, tag="rsq_r", name=f"rr_{nm}")
                nc.vector.reciprocal(r_[:], s_[:])
                t1_ = work.tile(shape, F32, tag="rsq_t1", name=f"rt_{nm}")
                nc.vector.tensor_tensor(t1_[:], r_[:], r_[:], ALU.mult)
                nc.vector.tensor_tensor(t1_[:], t1_[:], m_ap, ALU.mult)
                nc.vector.tensor_scalar(t1_[:], t1_[:], -0.5, 1.5, ALU.mult, ALU.add)
                y_ = work.tile(shape, F32, tag="rsq_y", name=f"ry_{nm}")
                nc.vector.tensor_tensor(y_[:], r_[:], t1_[:], ALU.mult)
                return y_

            s_x = rsqrt_refined(m_t[:], [128, TPC], "x")
            xn = big.tile([128, DB, TPC], BF16, tag="A", name="xn")
            nc.vector.tensor_tensor(
                xn[:], xw[:], ln1_sb[:, :, None].to_broadcast([128, DB, TPC]), ALU.mult
            )
            nc.vector.tensor_tensor(
                xn[:], xn[:], s_x[:, None, :].to_broadcast([128, DB, TPC]), ALU.mult
            )

            # ======== Q/K/V projections (natural layout) ========
            q_nat = big.tile([128, QT, H * HD], BF16, tag="B", name="q_nat")
            k_nat = work.tile([128, QT, KVH * HD], BF16, tag="k_nat", name="k_nat")
            v_nat = work.tile([128, QT, KVH * HD], BF16, tag="v_nat", name="v_nat")
            for (wT, dst, nout) in ((wqT, q_nat, H * HD), (wkT, k_nat, KVH * HD),
                                    (wvT, v_nat, KVH * HD)):
                for nch in range(nout // 512):
                    pq0 = ps.tile([128, 512], F32, tag="ps512", name="pq0")
                    pq1 = ps.tile([128, 512], F32, tag="ps512", name="pq1")
                    for b in range(DB):
                        wtile = wload.tile([128, 512], BF16, tag="w_qkv", name="wtile")
                        nc.sync.dma_start(
                            wtile[:],
                            wT.rearrange("(b p) n -> p b n", p=128)[:, b, ts(nch, 512)],
                        )
                        nc.tensor.matmul(pq0[:], xn[:, b, ts(0, 128)], wtile[:],
                                         start=(b == 0), stop=(b == DB - 1))
                        nc.tensor.matmul(pq1[:], xn[:, b, ts(1, 128)], wtile[:],
                                         start=(b == 0), stop=(b == DB - 1))
                    nc.vector.tensor_copy(dst[:, 0, ts(nch, 512)], pq0[:])
                    nc.vector.tensor_copy(dst[:, 1, ts(nch, 512)], pq1[:])

            # ======== qk-norm + rope (in natural layout, in-place) ========
            def qknorm_rope(z, nh, w_sb):
                z3 = z[:].rearrange("p q (h d) -> p q h d", h=nh)
                HG = 2
                for qt in range(QT):
                    for hg in range(nh // HG):
                        zz = z3[:, qt, ts(hg, HG)]
                        sqz = work.tile([128, HG, HD], F32, tag="qk_sq", name="sqz")
                        nc.vector.tensor_tensor(sqz[:], zz, zz, ALU.mult)
                        ss = work.tile([128, HG, 1], F32, tag="qk_ss", name="ss")
                        nc.vector.reduce_sum(ss[:], sqz[:], axis=AX.X)
                        nc.vector.tensor_scalar(ss[:], ss[:], 1.0 / HD, EPS, ALU.mult, ALU.add)
                        sc_ = rsqrt_refined(ss[:], [128, HG, 1], f"qk{nh}")
                        nc.vector.tensor_tensor(zz, zz, sc_[:].to_broadcast([128, HG, HD]), ALU.mult)
                        nc.vector.tensor_tensor(
                            zz, zz, w_sb[:, None, :].to_broadcast([128, HG, HD]), ALU.mult
                        )
                        cosb = cos_sb[:, qt, None, :].to_broadcast([128, HG, HD // 2])
                        sinb = sin_sb[:, qt, None, :].to_broadcast([128, HG, HD // 2])
                        z1 = zz[:, :, : HD // 2]
                        z2 = zz[:, :, HD // 2 :]
                        a_ = work.tile([128, HG, HD // 2], F32, tag="rope_a", name="ra")
                        b_ = work.tile([128, HG, HD // 2], F32, tag="rope_b", name="rb")
                        c_ = work.tile([128, HG, HD // 2], F32, tag="rope_c", name="rc")
                        d_ = work.tile([128, HG, HD // 2], F32, tag="rope_d", name="rd")
                        nc.vector.tensor_tensor(a_[:], z1, cosb, ALU.mult)
                        nc.vector.tensor_tensor(b_[:], z2, sinb, ALU.mult)
                        nc.vector.tensor_tensor(c_[:], z2, cosb, ALU.mult)
                        nc.vector.tensor_tensor(d_[:], z1, sinb, ALU.mult)
                        nc.vector.tensor_tensor(z1, a_[:], b_[:], ALU.subtract)
                        nc.vector.tensor_tensor(z2, c_[:], d_[:], ALU.add)

            qknorm_rope(q_nat, H, qnw_sb)
            qknorm_rope(k_nat, KVH, knw_sb)

            if debug:
                nc.sync.dma_start(dbg_q.rearrange("(q p) n -> p q n", p=128), q_nat[:])
                nc.sync.dma_start(dbg_k.rearrange("(q p) n -> p q n", p=128), k_nat[:])

            # ======== KV AllGather ========
            for qt in range(QT):
                nc.sync.dma_start(ag_kv_in[ts(qt, 128), :512], k_nat[:, qt])
                nc.sync.dma_start(ag_kv_in[ts(qt, 128), 512:], v_nat[:, qt])
            nc.gpsimd.collective_compute(
                "AllGather", ALU.bypass, replica_groups=[list(range(NCORE))],
                ins=[ag_kv_in[:].opt()], outs=[ag_kv_out[:].opt()],
            )

            # K^T transposes + V bf16, k-blocks reordered [evens | odds]
            kT = big.tile([128, KVH, 16, 128], BF16, tag="K", name="kT")
            v_bf = big.tile([128, 16, KVH * HD], BF16, tag="V", name="v_bf")
            for j, cb in enumerate(KORD):
                kv_sb = work.tile([128, 1024], BF16, tag="kv_sb", name="kv_sb")
                nc.sync.dma_start(kv_sb[:], ag_kv_out[ts(cb, 128)])
                nc.vector.tensor_copy(v_bf[:, j], kv_sb[:, 512:])
                for hk in range(KVH):
                    ptp = ps.tile([128, 128], BF16, tag="pstp", name="ktp")
                    nc.tensor.transpose(ptp[:], kv_sb[:, ts(hk, 128)], ident_b[:])
                    nc.vector.tensor_copy(kT[:, hk, j], ptp[:])

            # Q^T transposes
            qT = big.tile([128, QT, H, 128], BF16, tag="Q", name="qT")
            for qt in range(QT):
                for h in range(H):
                    ptp = ps.tile([128, 128], BF16, tag="pstp", name="qtp")
                    nc.tensor.transpose(ptp[:], q_nat[:, qt, ts(h, 128)], ident_b[:])
                    nc.vector.tensor_copy(qT[:, qt, h], ptp[:])

            # ======== attention ========
            o_nat = big.tile([128, QT, H * HD], F32, tag="A", name="o_nat")
            for h in range(H):
                hk = h // (H // KVH)
                for qt in range(QT):
                    # chunks of 256 k-tokens over reordered blocks
                    if qt == 0:
                        chunks = [(2 * j, j) for j in range(4)]            # evens, masked
                    else:
                        chunks = [(2 * j, None) for j in range(4)] + \
                                 [(8 + 2 * j, j) for j in range(4)]        # evens free, odds masked
                    ncks = len(chunks)
                    den = work.tile([128, 8], F32, tag="den", name="den")
                    o_ps = ps_acc.tile([128, 128], F32, tag="o_acc", name="o_ps")
                    for ci, (jb, mi) in enumerate(chunks):
                        s_ps = ps.tile([128, 256], F32, tag="ps512", name="s_ps")
                        nc.tensor.matmul(
                            s_ps[:], qT[:, qt, h],
                            kT[:, hk].rearrange("p b k -> p (b k)")[:, ds(jb * 128, 256)],
                            start=True, stop=True,
                        )
                        p_bf = work.tile([128, 256], BF16, tag="p_bf", name="p_bf")
                        if mi is None:
                            nc.scalar.activation(p_bf[:], s_ps[:], ACTF.Exp,
                                                 scale=SCALE, accum_out=den[:, ci : ci + 1])
                        else:
                            nc.scalar.activation(p_bf[:], s_ps[:], ACTF.Exp, scale=SCALE)
                            nc.vector.scalar_tensor_tensor(
                                p_bf[:], p_bf[:], 1.0, mask_sb[:, mi], ALU.mult, ALU.mult,
                                accum_out=den[:, ci : ci + 1],
                            )
                        for half in range(2):
                            pt_ps = ps.tile([128, 128], BF16, tag="pstp", name="pt_ps")
                            nc.tensor.transpose(pt_ps[:], p_bf[:, ts(half, 128)], ident_b[:])
                            pt_sb = work.tile([128, 128], BF16, tag="pt_sb", name="pt_sb")
                            nc.vector.tensor_copy(pt_sb[:], pt_ps[:])
                            nc.tensor.matmul(
                                o_ps[:], pt_sb[:], v_bf[:, jb + half, ts(hk, 128)],
                                start=(ci == 0 and half == 0),
                                stop=(ci == ncks - 1 and half == 1),
                            )
                    dsum = work.tile([128, 1], F32, tag="dsum", name="dsum")
                    nc.vector.reduce_sum(dsum[:], den[:, :ncks], axis=AX.X)
                    drec = work.tile([128, 1], F32, tag="drec", name="drec")
                    nc.vector.reciprocal(drec[:], dsum[:])
                    nc.vector.tensor_scalar(o_nat[:, qt, ts(h, 128)], o_ps[:], drec[:],
                                            None, ALU.mult)

            if debug:
                nc.sync.dma_start(dbg_o.rearrange("(q p) n -> p q n", p=128), o_nat[:])

            # ======== o^T + O-projection + residual -> h1^T ========
            oT = big.tile([128, H, QT, 128], BF16, tag="O", name="oT")
            for qt in range(QT):
                for hb in range(H):
                    ptp = ps.tile([128, 128], F32, tag="pstp", name="otp")
                    nc.tensor.transpose(ptp[:], o_nat[:, qt, ts(hb, 128)], ident_f[:])
                    nc.vector.tensor_copy(oT[:, hb, qt], ptp[:])
            h1T = big.tile([128, DB, TPC], F32, tag="H", name="h1T")
            for db in range(DB):
                wo_col = wload.tile([128, H, 128], BF16, tag="wo_col", name="wo_col")
                nc.sync.dma_start(
                    wo_col[:], woT.rearrange("(b p) n -> p b n", p=128)[:, :, ts(db, 128)]
                )
                ph = ps.tile([128, TPC], F32, tag="ps512", name="ph")
                for hb in range(H):
                    nc.tensor.matmul(
                        ph[:], wo_col[:, hb],
                        oT[:, hb].rearrange("p q t -> p (q t)"),
                        start=(hb == 0), stop=(hb == H - 1),
                    )
                nc.vector.tensor_tensor(h1T[:, db], ph[:], xw[:, db], ALU.add)

            if debug:
                nc.sync.dma_start(dbg_h1T.rearrange("(b p) t -> p b t", p=128), h1T[:])

            # ======== t = rms(h1)*ln2 (bf16), gate from h1T (fp32) ========
            ssq2 = ps_acc.tile([128, TPC], F32, tag="ssq", name="ssq2")
            for b in range(DB):
                sqb = work.tile([128, TPC], F32, tag="sqb", name="sqb2")
                nc.vector.tensor_tensor(sqb[:], h1T[:, b], h1T[:, b], ALU.mult)
                nc.tensor.matmul(ssq2[:], ones_f[:], sqb[:], start=(b == 0), stop=(b == DB - 1))
            m2 = work.tile([128, TPC], F32, tag="m_rms", name="m2")
            nc.vector.tensor_scalar(m2[:], ssq2[:], 1.0 / D, EPS, ALU.mult, ALU.add)
            s_t = rsqrt_refined(m2[:], [128, TPC], "t")
            tT_bf = big.tile([128, DB, TPC], BF16, tag="T", name="tT_bf")
            for b in range(DB):
                nc.vector.scalar_tensor_tensor(
                    tT_bf[:, b], h1T[:, b], ln2_sb[:, b : b + 1], s_t[:],
                    ALU.mult, ALU.mult,
                )

            # t natural (bf16) -> AllGather (dense-FFN source)
            t_nat = big.tile([128, QT, D], BF16, tag="V", name="t_nat")
            for qt in range(QT):
                for db in range(DB):
                    ptp = ps.tile([128, 128], BF16, tag="pstp", name="ttp")
                    nc.tensor.transpose(ptp[:], tT_bf[:, db, ts(qt, 128)], ident_b[:])
                    nc.vector.tensor_copy(t_nat[:, qt, ts(db, 128)], ptp[:])
            nc.sync.dma_start(ag_t_in.rearrange("(q p) d -> p q d", p=128), t_nat[:])
            nc.gpsimd.collective_compute(
                "AllGather", ALU.bypass, replica_groups=[list(range(NCORE))],
                ins=[ag_t_in[:].opt()], outs=[t_full[:].opt()],
            )
            if debug:
                nc.sync.dma_start(dbg_t.rearrange("(q p) d -> p q d", p=128), t_nat[:])

            # gate scores: z = (gate_w*ln2)^T h1 scaled per-token, sigmoid
            gw2 = cpool.tile([128, DB, E], F32)
            nc.vector.tensor_tensor(
                gw2[:], gwT_sb[:], ln2_sb[:, :, None].to_broadcast([128, DB, E]), ALU.mult
            )
            sc_nat = work.tile([128, QT, E], F32, tag="sc_nat", name="sc_nat")
            for qt in range(QT):
                pz = ps.tile([128, E], F32, tag="pstp", name="pz")
                for b in range(DB):
                    nc.tensor.matmul(pz[:], h1T[:, b, ts(qt, 128)], gw2[:, b],
                                     start=(b == 0), stop=(b == DB - 1))
                stp = ps.tile([128, 128], F32, tag="pstp", name="stp")
                nc.tensor.transpose(stp[:, :1], s_t[:1, ts(qt, 128)], ident_f[:1, :1])
                scol = work.tile([128, 1], F32, tag="scol", name="scol")
                nc.vector.tensor_copy(scol[:], stp[:, :1])
                nc.scalar.activation(sc_nat[:, qt], pz[:], ACTF.Sigmoid, scale=scol[:])
            nc.sync.dma_start(ag_s_in.rearrange("(q p) e -> p q e", p=128), sc_nat[:])
            nc.gpsimd.collective_compute(
                "AllGather", ALU.bypass, replica_groups=[list(range(NCORE))],
                ins=[ag_s_in[:].opt()], outs=[sc_full[:].opt()],
            )

            # ======== routing (replicated over all 2048 tokens) ========
            sc_all = route.tile([128, 16, E], F32, tag="sc_all", name="sc_all")
            nc.sync.dma_start(sc_all[:], sc_full.rearrange("(tb p) e -> p tb e", p=128))
            C_all = route.tile([128, 16, E], F32, tag="C_all", name="C_all")
            CeA = [route.tile([128, 16], F32, tag=f"CeA{k}", name=f"CeA{k}") for k in range(2)]
            for tb in range(16):
                sc_tb = sc_all[:, tb]
                s2 = route.tile([128, E], F32, tag="s2", name="s2")
                nc.vector.tensor_tensor(s2[:], sc_tb, gb_sb[:], ALU.add)
                gs = route.tile([128, G], F32, tag="gs", name="gs")
                grp_pad = route.tile([128, 8], F32, tag="grp_pad", name="grp_pad")
                for g in range(G):
                    nc.vector.memset(grp_pad[:], -1e30)
                    nc.vector.tensor_copy(grp_pad[:, :4], s2[:, ts(g, 4)])
                    mx8 = route.tile([128, 8], F32, tag="mx8", name="mx8")
                    mi8 = route.tile([128, 8], U32, tag="mi8", name="mi8")
                    nc.vector.max_with_indices(mx8[:], mi8[:], grp_pad[:])
                    nc.vector.tensor_tensor(gs[:, g : g + 1], mx8[:, 0:1], mx8[:, 1:2], ALU.add)
                gpad2 = route.tile([128, 8], F32, tag="gpad2", name="gpad2")
                nc.vector.memset(gpad2[:], -1e30)
                nc.vector.tensor_copy(gpad2[:, :4], gs[:])
                gv8 = route.tile([128, 8], F32, tag="gv8", name="gv8")
                gi8 = route.tile([128, 8], U32, tag="gi8", name="gi8")
                nc.vector.max_with_indices(gv8[:], gi8[:], gpad2[:])
                gmask = route.tile([128, G], F32, tag="gmask", name="gmask")
                nc.vector.tensor_scalar(gmask[:], gs[:], gv8[:, 1:2], None, ALU.is_ge)
                masked = route.tile([128, E], F32, tag="masked", name="masked")
                nc.vector.tensor_tensor(
                    masked[:].rearrange("p (g e) -> p g e", g=G),
                    s2[:].rearrange("p (g e) -> p g e", g=G),
                    gmask[:, :, None].to_broadcast([128, G, E // G]),
                    ALU.mult,
                )
                mv = route.tile([128, 8], F32, tag="mv", name="mv")
                mi = route.tile([128, 8], U32, tag="mi", name="mi")
                nc.vector.max_with_indices(mv[:], mi[:], masked[:])
                mi_f = route.tile([128, TOPK], F32, tag="mi_f", name="mi_f")
                nc.vector.tensor_copy(mi_f[:], mi[:, :TOPK])
                w4 = route.tile([128, TOPK], F32, tag="w4", name="w4")
                ohs = []
                for j in range(TOPK):
                    oh = route.tile([128, E], F32, tag=f"oh{j}", name=f"oh{j}")
                    nc.vector.tensor_scalar(oh[:], iota16[:], mi_f[:, j : j + 1], None,
                                            ALU.is_equal)
                    scratch = route.tile([128, E], F32, tag="scr", name="scr")
                    nc.vector.scalar_tensor_tensor(
                        scratch[:], oh[:], 1.0, sc_tb, ALU.mult, ALU.mult,
                        accum_out=w4[:, j : j + 1],
                    )
                    ohs.append(oh)
                wsum = route.tile([128, 1], F32, tag="wsum", name="wsum")
                nc.vector.reduce_sum(wsum[:], w4[:], axis=AX.X)
                nc.vector.tensor_scalar(wsum[:], wsum[:], 1e-20, None, ALU.add)
                winv = route.tile([128, 1], F32, tag="winv", name="winv")
                nc.vector.reciprocal(winv[:], wsum[:])
                wn4 = route.tile([128, TOPK], F32, tag="wn4", name="wn4")
                nc.vector.tensor_scalar(wn4[:], w4[:], winv[:], RSF, ALU.mult, ALU.mult)
                C_tb = C_all[:, tb]
                nc.vector.tensor_scalar(C_tb, ohs[0][:], wn4[:, 0:1], None, ALU.mult)
                for j in range(1, TOPK):
                    nc.vector.scalar_tensor_tensor(
                        C_tb, ohs[j][:], wn4[:, j : j + 1], C_tb, ALU.mult, ALU.add
                    )
                for k in range(2):
                    scr2 = route.tile([128, E], F32, tag="scr2", name="scr2")
                    nc.vector.scalar_tensor_tensor(
                        scr2[:], C_tb, 1.0, esel_sb[k][:, :E], ALU.mult, ALU.mult,
                        accum_out=CeA[k][:, tb : tb + 1],
                    )

            if debug:
                nc.sync.dma_start(dbg_C.rearrange("(tb p) e -> p tb e", p=128), C_all[:])

            # ======== dense expert FFN (2 experts over all tokens) ========
            # process tokens in 2 chunks of 1024; everything plain-DMA
            TCH = 512
            for tch in range(L // TCH):
                # t_full^T for this token chunk (PE transposes of the AG output)
                tfT = big.tile([128, DB, TCH], BF16, tag="K", name=f"tfT{tch}")
                for pb in range(TCH // 128):
                    trow = work.tile([128, D], BF16, tag="kv_sb", name="trow")
                    nc.sync.dma_start(trow[:], t_full[ts(tch * (TCH // 128) + pb, 128)])
                    for db in range(DB):
                        ptp = ps.tile([128, 128], BF16, tag="pstp", name="ttp2")
                        nc.tensor.transpose(ptp[:], trow[:, ts(db, 128)], ident_b[:])
                        nc.vector.tensor_copy(tfT[:, db, ts(pb, 128)], ptp[:])
                hm2 = []
                for k in range(2):
                    hm = big.tile([128, I_FF // 128, TCH], BF16, tag=("O" if k == 0 else "Q"),
                                  name=f"hm{k}_{tch}")
                    for mat_i, mat in enumerate((wg_p, wu_p)):
                        for ich in range(I_FF // 128):
                            wmat = wload.tile([128, DB, 128], BF16, tag="w_exp", name="wmat")
                            nc.sync.dma_start(
                                wmat[:],
                                mat.rearrange("e (b p) i -> p e b i", p=128)[:, k, :, ts(ich, 128)],
                            )
                            for nch in range(TCH // 512):
                                pg = ps.tile([128, 512], F32, tag="ps512", name="pg")
                                for b in range(DB):
                                    nc.tensor.matmul(
                                        pg[:], wmat[:, b], tfT[:, b, ts(nch, 512)],
                                        start=(b == 0), stop=(b == DB - 1),
                                    )
                                if mat_i == 0:
                                    nc.scalar.activation(hm[:, ich, ts(nch, 512)],
                                                         pg[:], ACTF.Silu)
                                else:
                                    nc.vector.tensor_tensor(
                                        hm[:, ich, ts(nch, 512)],
                                        hm[:, ich, ts(nch, 512)], pg[:], ALU.mult,
                                    )
                    hm2.append(hm)
                # down projection for both experts, combine with per-token weights
                for dch in range(D // 256):
                    wd0 = wload.tile([128, I_FF // 128, 256], BF16, tag="wd0", name="wd0")
                    nc.sync.dma_start(
                        wd0[:], wd_p.rearrange("e (b p) d -> p e b d", p=128)[:, 0, :, ts(dch, 256)]
                    )
                    wd1 = wload.tile([128, I_FF // 128, 256], BF16, tag="wd1", name="wd1")
                    nc.sync.dma_start(
                        wd1[:], wd_p.rearrange("e (b p) d -> p e b d", p=128)[:, 1, :, ts(dch, 256)]
                    )
                    for tb in range(TCH // 128):
                        tbg = tch * (TCH // 128) + tb
                        py0 = ps_acc.tile([128, 256], F32, tag="pd0", name="py0")
                        for ib in range(I_FF // 128):
                            nc.tensor.matmul(py0[:], hm2[0][:, ib, ts(tb, 128)], wd0[:, ib],
                                             start=(ib == 0), stop=(ib == I_FF // 128 - 1))
                        py1 = ps_acc.tile([128, 256], F32, tag="pd1", name="py1")
                        for ib in range(I_FF // 128):
                            nc.tensor.matmul(py1[:], hm2[1][:, ib, ts(tb, 128)], wd1[:, ib],
                                             start=(ib == 0), stop=(ib == I_FF // 128 - 1))
                        ycmb = work.tile([128, 256], F32, tag="ycmb", name="ycmb")
                        nc.vector.tensor_scalar(ycmb[:], py0[:], CeA[0][:, tbg : tbg + 1],
                                                None, ALU.mult)
                        nc.vector.scalar_tensor_tensor(
                            ycmb[:], py1[:], CeA[1][:, tbg : tbg + 1], ycmb[:],
                            ALU.mult, ALU.add,
                        )
                        ybf = work.tile([128, 256], BF16, tag="p_bf", name="ybf")
                        nc.vector.tensor_copy(ybf[:], ycmb[:])
                        nc.sync.dma_start(y_dram[ts(tbg, 128), ts(dch, 256)], ybf[:])

            # ======== ReduceScatter y ========
            nc.gpsimd.collective_compute(
                "ReduceScatter", ALU.add, replica_groups=[list(range(NCORE))],
                ins=[y_dram[:].opt()], outs=[y_shard[:].opt()],
            )

            # ======== shared expert ========
            hms = big.tile([128, SH_I // 128, TPC], BF16, tag="Q", name="hms")
            for mat_i, mat in enumerate((shgT, shuT)):
                for shc in range(SH_I // 128):
                    wcol_sh = wload.tile([128, DB, 128], BF16, tag="w_exp", name="wcol_sh")
                    nc.sync.dma_start(
                        wcol_sh[:], mat.rearrange("(b p) n -> p b n", p=128)[:, :, ts(shc, 128)]
                    )
                    pgs = ps.tile([128, TPC], F32, tag="ps512", name="pgs")
                    for b in range(DB):
                        nc.tensor.matmul(pgs[:], wcol_sh[:, b], tT_bf[:, b],
                                         start=(b == 0), stop=(b == DB - 1))
                    if mat_i == 0:
                        nc.scalar.activation(hms[:, shc], pgs[:], ACTF.Silu)
                    else:
                        nc.vector.tensor_tensor(hms[:, shc], hms[:, shc], pgs[:], ALU.mult)

            # ======== final: out = h1 + y + sh (natural layout) ========
            y_sh_sb = big.tile([128, QT, D], BF16, tag="C", name="y_sh_sb")
            nc.sync.dma_start(y_sh_sb[:], y_shard.rearrange("(q p) d -> p q d", p=128))
            if debug:
                nc.sync.dma_start(dbg_y.rearrange("(q p) d -> p q d", p=128), y_sh_sb[:])
            h1n = big.tile([128, QT, D], BF16, tag="A", name="h1n")
            for qt in range(QT):
                for db in range(DB):
                    ptp = ps.tile([128, 128], F32, tag="pstp", name="h1tp")
                    nc.tensor.transpose(ptp[:], h1T[:, db, ts(qt, 128)], ident_f[:])
                    nc.vector.tensor_copy(h1n[:, qt, ts(db, 128)], ptp[:])
            for dch in range(D // 128):
                wdcol = wload.tile([128, SH_I // 128, 128], BF16, tag="w_exp", name="wdcol")
                nc.sync.dma_start(
                    wdcol[:], shdT.rearrange("(b p) d -> p b d", p=128)[:, :, ts(dch, 128)]
                )
                for qt in range(QT):
                    psh = ps.tile([128, 128], F32, tag="pstp", name="psh")
                    for sb_ in range(SH_I // 128):
                        nc.tensor.matmul(psh[:], hms[:, sb_, ts(qt, 128)], wdcol[:, sb_],
                                         start=(sb_ == 0), stop=(sb_ == SH_I // 128 - 1))
                    oc = work.tile([128, 128], F32, tag="ycmb", name="oc")
                    nc.vector.tensor_tensor(oc[:], psh[:], h1n[:, qt, ts(dch, 128)], ALU.add)
                    nc.vector.tensor_tensor(oc[:], oc[:], y_sh_sb[:, qt, ts(dch, 128)], ALU.add)
                    nc.sync.dma_start(
                        out_sh.rearrange("(q p) d -> p q d", p=128)[:, qt, ts(dch, 128)],
                        oc[:],
                    )

    return nc


# =====================================================================
# host side
# =====================================================================

_BUILD_CACHE = {}


def _get_nc(debug=False):
    if debug not in _BUILD_CACHE:
        nc = build(debug=debug)
        _fixup_multi_waits(nc)
        _BUILD_CACHE[debug] = nc
    return _BUILD_CACHE[debug]


def _prep_inputs(inputs):
    bf = ml_dtypes.bfloat16
    x = np.asarray(inputs["x"], np.float32).reshape(L, D)
    x_perm = x[PERM]
    wq, wk, wv, wo = (np.asarray(inputs[k], np.float32) for k in ("w_q", "w_k", "w_v", "w_o"))
    gate_w = np.asarray(inputs["gate_w"], np.float32)
    gate_bias = np.asarray(inputs["gate_bias"], np.float32)
    wg, wu, wd = (np.asarray(inputs[k], np.float32) for k in ("wg", "wu", "wd"))
    shg, shu, shd = (np.asarray(inputs[k], np.float32) for k in ("sh_g", "sh_u", "sh_d"))

    pos = np.arange(L, dtype=np.float32)
    inv = 1.0 / (THETA ** (np.arange(0, HD, 2, dtype=np.float32) / HD))
    ang = pos[:, None] * inv[None, :]
    cos_t, sin_t = np.cos(ang).astype(np.float32), np.sin(ang).astype(np.float32)

    common = {
        "wqT": np.ascontiguousarray(wq.T).astype(bf),
        "wkT": np.ascontiguousarray(wk.T).astype(bf),
        "wvT": np.ascontiguousarray(wv.T).astype(bf),
        "woT": np.ascontiguousarray(wo.T).astype(bf),
        "gwT": np.ascontiguousarray(gate_w.T),
        "gbias": gate_bias.reshape(1, E).astype(np.float32),
        "ln1pd": np.asarray(inputs["ln1_w"], np.float32).reshape(DB, 128).T.copy(),
        "ln2pd": np.asarray(inputs["ln2_w"], np.float32).reshape(DB, 128).T.copy(),
        "qnw": np.asarray(inputs["q_norm_w"], np.float32).reshape(1, HD),
        "knw": np.asarray(inputs["k_norm_w"], np.float32).reshape(1, HD),
        "shgT": np.ascontiguousarray(shg.T).astype(bf),
        "shuT": np.ascontiguousarray(shu.T).astype(bf),
        "shdT": np.ascontiguousarray(shd.T).astype(bf),
    }

    in_maps = []
    tri = np.tril(np.ones((128, 128), np.float32), -1)
    for c in range(NCORE):
        sl = slice(c * TPC, (c + 1) * TPC)
        abs_tok = PERM[sl]
        e0, e1 = PAIRS[c]
        es0 = np.zeros((1, CPAD), np.float32)
        es0[0, e0] = 1.0
        es1 = np.zeros((1, CPAD), np.float32)
        es1[0, e1] = 1.0
        # allowed(i, p, j) = j < p or (j == p and i <= c); pairs (2u, 2u+1)
        m = np.zeros((8, 128, 128), np.float32)
        for i in range(8):
            m[i] = tri + np.eye(128, dtype=np.float32) * (1.0 if i <= c else 0.0)
        mp = np.concatenate([m[::2], m[1::2]], axis=0)  # no-op order helper
        maskpair = np.zeros((4, 128, 256), np.float32)
        for u in range(4):
            maskpair[u, :, :128] = m[2 * u]
            maskpair[u, :, 128:] = m[2 * u + 1]
        im = {
            **common,
            "xT32": np.ascontiguousarray(x_perm[sl].T),
            "cosq": cos_t[abs_tok],
            "sinq": sin_t[abs_tok],
            "maskp": maskpair.astype(bf),
            "esel0": es0,
            "esel1": es1,
            "wg_p": np.ascontiguousarray(wg[[e0, e1]].transpose(0, 2, 1)).astype(bf),
            "wu_p": np.ascontiguousarray(wu[[e0, e1]].transpose(0, 2, 1)).astype(bf),
            "wd_p": np.ascontiguousarray(wd[[e0, e1]].transpose(0, 2, 1)).astype(bf),
        }
        in_maps.append(im)
    return in_maps


def run(inputs, debug=False):
    nc = _get_nc(debug=debug)
    in_maps = _prep_inputs(inputs)
    return run_bass_kernel_spmd(nc, in_maps, core_ids=list(range(NCORE)))


def kernel(**inputs) -> np.ndarray:
    res = run(inputs, debug=False)
    out_perm = np.concatenate([res.results[c]["out_sh"] for c in range(NCORE)], axis=0)
    out = np.empty((L, D), np.float32)
    out[PERM] = out_perm
    return out.reshape(1, L, D)


# revision 33
# speedup vs baseline: 1.0103x; 1.0103x over previous
"""Trainium2 Bass kernel for nn_DecoderLayer_43963285242628.

Decoder layer: RMSNorm -> GQA attention (QK-norm + split-half RoPE, causal)
-> residual -> RMSNorm -> MoE (16 experts, group-limited top-4 sigmoid
routing) + shared SwiGLU expert -> residual.

Distribution over 8 NeuronCores:
- Tokens are striped over cores (permuted position c*256+t <-> abs token
  c+8t) so causal-attention work per core is identical and the whole
  program is static (causality handled by per-core data masks).
- Attention + shared expert: token-parallel (each core its 256 tokens).
- MoE FFN: expert-parallel (2 experts/core): AllGather of t, routing
  replicated on-device, token dispatch via matmul prefix-sum compaction +
  dma_scatter_add/dma_gather, bf16 ReduceScatter of expert outputs.
- Precision: attention chain f32r (tf32-grade; routing needs an accurate
  gate), probabilities/AV bf16, expert + shared FFN bf16, residuals fp32.
"""

import numpy as np
import ml_dtypes

import bass_rust
import concourse.bass as bass
import concourse.mybir as mybir
import concourse.tile as tile
from concourse.bass import ts, ds
from concourse.bass_utils import run_bass_kernel_spmd
from concourse.masks import make_identity

F32 = mybir.dt.float32
F32R = mybir.dt.float32r
BF16 = mybir.dt.bfloat16
I16 = mybir.dt.int16
U32 = mybir.dt.uint32
AX = mybir.AxisListType
ALU = bass.mybir.AluOpType
ACTF = mybir.ActivationFunctionType

# ---- problem constants ----
NCORE = 8
L, D = 2048, 2048
H, KVH, HD = 16, 4, 128
E, TOPK, G, TKG = 16, 4, 4, 2
I_FF, SH_I = 1024, 2048
EPS = 1e-5
THETA = 1e6
SCALE = HD ** -0.5
RSF = 2.5

TPC = L // NCORE          # 256 tokens per core
QT = TPC // 128           # 2 q-tiles
DB = D // 128             # 16
CAP = 768                 # per-expert token capacity
CAPB = CAP // 128         # 6
CPAD = 64

PAIRS = [(11, 1), (10, 15), (9, 14), (12, 13), (8, 5), (0, 2), (6, 7), (3, 4)]
PERM = np.array([c + NCORE * t for c in range(NCORE) for t in range(TPC)])
# k-block order in SBUF: even permuted blocks first, then odd
KORD = [2 * i for i in range(8)] + [2 * i + 1 for i in range(8)]

_MAXW = 1
_uid = [0]


def _patched_drain_and_barrier(self, tick_clock, wait_clock):
    # this walrus build rejects >1 sem wait per instruction; spill the
    # kernel-tail drain's waits onto follow-up SP nops
    nc = self.nc
    drain_inst = nc.sync.drain()
    wait_clock.add_sem_waits(
        drain_inst.ins, tile.ScopedClock({None: tick_clock.global_clock})
    )
    si = drain_inst.ins.sync_info
    waits = list(si.on_wait) if si is not None else []
    if len(waits) > _MAXW:
        si.on_wait = waits[:_MAXW]
        drain_inst.ins.sync_info = si
        for w in waits[_MAXW:]:
            nop = nc.sync.nop(hint="drain_wait_spill", nofuse=True)
            nop.ins.sync_info = bass_rust.SyncInfo(on_wait=[w], on_update=[])
    nc.all_engine_barrier()
    assert self.sems is not None
    popped = nc._tile_sem_poison_stack.pop()
    assert popped is self._sem_poison
    nc.clear_and_free_semaphores(list(self.sems.allocated().values()))
    nc.all_engine_barrier()


tile.TileContext._drain_and_barrier = _patched_drain_and_barrier


def _fixup_multi_waits(nc):
    """Split multi-wait instructions: extras go on inserted same-engine NoOps."""
    n_split = 0
    for fn in nc.m.functions:
        for bb in fn.blocks:
            il = bb.instructions
            i = 0
            while i < len(il):
                ins = il[i]
                si = ins.sync_info
                if si is None:
                    i += 1
                    continue
                waits = list(si.on_wait)
                if len(waits) <= 1:
                    i += 1
                    continue
                si.on_wait = waits[-1:]
                ins.sync_info = si
                for w in waits[:-1]:
                    _uid[0] += 1
                    nop = mybir.InstNoOp(name=f"I-waitspill-{_uid[0]}", ins=[], outs=[])
                    nop.engine = ins.engine
                    nop.sync_info = bass_rust.SyncInfo(on_wait=[w], on_update=[])
                    il.insert(i, nop)
                    i += 1
                n_split += 1
                i += 1
    return n_split


def build(debug=False):
    nc = bass.Bass()
    dp = nc.declare_dram_parameter

    # ---------------- per-core inputs ----------------
    xT32 = dp("xT32", [D, TPC], F32, isOutput=False)
    wqT = dp("wqT", [D, H * HD], BF16, isOutput=False)
    wkT = dp("wkT", [D, KVH * HD], BF16, isOutput=False)
    wvT = dp("wvT", [D, KVH * HD], BF16, isOutput=False)
    woT = dp("woT", [H * HD, D], BF16, isOutput=False)
    gwT = dp("gwT", [D, E], F32R, isOutput=False)
    gbias = dp("gbias", [1, E], F32, isOutput=False)
    ln1pd = dp("ln1pd", [128, DB], F32, isOutput=False)
    ln2pd = dp("ln2pd", [128, DB], F32, isOutput=False)
    qnw = dp("qnw", [1, HD], F32, isOutput=False)
    knw = dp("knw", [1, HD], F32, isOutput=False)
    cosq = dp("cosq", [TPC, HD // 2], F32, isOutput=False)
    sinq = dp("sinq", [TPC, HD // 2], F32, isOutput=False)
    maskp = dp("maskp", [4, 128, 256], BF16, isOutput=False)   # paired causal masks
    esel0 = dp("esel0", [1, CPAD], F32, isOutput=False)
    esel1 = dp("esel1", [1, CPAD], F32, isOutput=False)
    wg_p = dp("wg_p", [2, D, I_FF], BF16, isOutput=False)
    wu_p = dp("wu_p", [2, D, I_FF], BF16, isOutput=False)
    wd_p = dp("wd_p", [2, I_FF, D], BF16, isOutput=False)
    shgT = dp("shgT", [D, SH_I], BF16, isOutput=False)
    shuT = dp("shuT", [D, SH_I], BF16, isOutput=False)
    shdT = dp("shdT", [SH_I, D], BF16, isOutput=False)

    # ---------------- outputs ----------------
    out_sh = dp("out_sh", [TPC, D], F32, isOutput=True)
    if debug:
        dbg_h1T = dp("dbg_h1T", [D, TPC], F32, isOutput=True)
        dbg_t = dp("dbg_t", [TPC, D], BF16, isOutput=True)
        dbg_C = dp("dbg_C", [L, E], F32, isOutput=True)
        dbg_q = dp("dbg_q", [TPC, H * HD], F32, isOutput=True)
        dbg_k = dp("dbg_k", [TPC, KVH * HD], F32, isOutput=True)
        dbg_o = dp("dbg_o", [TPC, H * HD], F32, isOutput=True)
        dbg_y = dp("dbg_y", [TPC, D], BF16, isOutput=True)

    with tile.TileContext(nc) as tc:
        with (
            tc.tile_pool(name="const", bufs=1) as cpool,
            tc.tile_pool(name="big", bufs=1) as big,
            tc.tile_pool(name="wload", bufs=2) as wload,
            tc.tile_pool(name="work", bufs=2) as work,
            tc.tile_pool(name="route", bufs=2) as route,
            tc.tile_pool(name="ps", bufs=2, space="PSUM") as ps,
            tc.tile_pool(name="ps_acc", bufs=1, space="PSUM") as ps_acc,
            tc.tile_pool(name="dram", bufs=1, space="DRAM") as dram,
        ):
            # ======== constants ========
            ident_f = cpool.tile([128, 128], F32)
            make_identity(nc, ident_f[:])
            ident_b = cpool.tile([128, 128], BF16)
            make_identity(nc, ident_b[:])
            ones_f = cpool.tile([128, 128], F32)
            nc.vector.memset(ones_f[:], 1.0)
            ones1 = cpool.tile([1, 128], F32)
            nc.vector.memset(ones1[:], 1.0)

            def bcast_row(row_ap, width, dst):
                pb = ps.tile([128, 512], F32, tag="ps512", name="pb")
                nc.tensor.matmul(pb[:, :width], ones1[:], row_ap, start=True, stop=True)
                nc.vector.tensor_copy(dst[:], pb[:, :width])

            ln1_sb = cpool.tile([128, DB], F32)
            nc.sync.dma_start(ln1_sb[:], ln1pd[:])
            ln2_sb = cpool.tile([128, DB], F32)
            nc.sync.dma_start(ln2_sb[:], ln2pd[:])
            qnw_sb = cpool.tile([128, HD], F32)
            qnw_row = cpool.tile([1, HD], F32)
            nc.sync.dma_start(qnw_row[:], qnw[:])
            bcast_row(qnw_row[:], HD, qnw_sb)
            knw_sb = cpool.tile([128, HD], F32)
            knw_row = cpool.tile([1, HD], F32)
            nc.sync.dma_start(knw_row[:], knw[:])
            bcast_row(knw_row[:], HD, knw_sb)
            cos_sb = cpool.tile([128, QT, HD // 2], F32)
            nc.sync.dma_start(cos_sb[:], cosq.rearrange("(q p) f -> p q f", p=128))
            sin_sb = cpool.tile([128, QT, HD // 2], F32)
            nc.sync.dma_start(sin_sb[:], sinq.rearrange("(q p) f -> p q f", p=128))
            mask_sb = cpool.tile([128, 4, 256], BF16)
            nc.sync.dma_start(mask_sb[:], maskp.rearrange("i p j -> p i j"))
            gb_row = cpool.tile([1, E], F32)
            nc.sync.dma_start(gb_row[:], gbias[:])
            gb_sb = cpool.tile([128, E], F32)
            bcast_row(gb_row[:], E, gb_sb)
            esel_sb = []
            for k, esel_p in enumerate((esel0, esel1)):
                row_ = cpool.tile([1, CPAD], F32, tag=f"eselr{k}", name=f"eselr{k}")
                nc.sync.dma_start(row_[:], esel_p[:])
                t_ = cpool.tile([128, CPAD], F32, tag=f"eselb{k}", name=f"eselb{k}")
                bcast_row(row_[:], CPAD, t_)
                esel_sb.append(t_)
            iota16 = cpool.tile([128, E], F32)
            nc.gpsimd.iota(iota16[:], pattern=[[1, E]], base=0, channel_multiplier=0,
                           allow_small_or_imprecise_dtypes=True)
            gwT_sb = cpool.tile([128, DB, E], F32R)
            nc.sync.dma_start(gwT_sb[:], gwT.rearrange("(b p) e -> p b e", p=128))

            # ======== DRAM internals ========
            ag_kv_in = dram.tile([TPC, 1024], BF16)
            ag_kv_out = dram.tile([L, 1024], BF16)
            ag_t_in = dram.tile([TPC, D], BF16)
            t_full = dram.tile([L, D], BF16)
            ag_s_in = dram.tile([TPC, E], F32)
            sc_full = dram.tile([L, E], F32)
            y_dram = dram.tile([L, D], BF16)
            y_shard = dram.tile([TPC, D], BF16)

            # ======== x^T load + rms -> xn (f32r) ========
            # big-pool slot tags (lifetime chains):
            #   A: xn -> o_nat      B: x_sb(2 gens) -> hm      C: q -> tT_r
            #   K: kT -> tg         V: v_bf -> out_f           Q: qT -> hms
            #   O: oT -> y_sb       H: h1T                     T: tT_bf -> y_sh
            xw = big.tile([128, DB, TPC], F32, tag="C", name="xw")
            nc.sync.dma_start(xw[:], xT32.rearrange("(b p) t -> p b t", p=128))
            ssq_ps = ps_acc.tile([128, TPC], F32, tag="pd0", name="ssq_ps")
            for b in range(DB):
                sqb = work.tile([128, TPC], F32, tag="sqb", name="sqb")
                nc.vector.tensor_tensor(sqb[:], xw[:, b], xw[:, b], ALU.mult)
                nc.tensor.matmul(ssq_ps[:], ones_f[:], sqb[:], start=(b == 0), stop=(b == DB - 1))
            m_t = work.tile([128, TPC], F32, tag="m_rms", name="m_t")
            nc.vector.tensor_scalar(m_t[:], ssq_ps[:], 1.0 / D, EPS, ALU.mult, ALU.add)
            # newton-refined rsqrt (ACT sqrt is loose)
            def rsqrt_refined(m_ap, shape, nm):
                s_ = work.tile(shape, F32, tag="rsq_s", name=f"rs_{nm}")
                nc.scalar.activation(s_[:], m_ap, ACTF.Sqrt)
                r_ = work.tile(shape, F32, tag="rsq_r", name=f"rr_{nm}")
                nc.vector.reciprocal(r_[:], s_[:])
                t1_ = work.tile(shape, F32, tag="rsq_t1", name=f"rt_{nm}")
                nc.vector.tensor_tensor(t1_[:], r_[:], r_[:], ALU.mult)
                nc.vector.tensor_tensor(t1_[:], t1_[:], m_ap, ALU.mult)
                nc.vector.tensor_scalar(t1_[:], t1_[:], -0.5, 1.5, ALU.mult, ALU.add)
                y_ = work.tile(shape, F32, tag="rsq_y", name=f"ry_{nm}")
                nc.vector.tensor_tensor(y_[:], r_[:], t1_[:], ALU.mult)
                return y_

            s_x = rsqrt_refined(m_t[:], [128, TPC], "x")
            xn = big.tile([128, DB, TPC], BF16, tag="A", name="xn")
            nc.vector.tensor_tensor(
                xn[:], xw[:], ln1_sb[:, :, None].to_broadcast([128, DB, TPC]), ALU.mult
            )
            nc.vector.tensor_tensor(
                xn[:], xn[:], s_x[:, None, :].to_broadcast([128, DB, TPC]), ALU.mult
            )

            # ======== Q/K/V projections (natural layout) ========
            q_nat = big.tile([128, QT, H * HD], BF16, tag="B", name="q_nat")
            k_nat = work.tile([128, QT, KVH * HD], BF16, tag="k_nat", name="k_nat")
            v_nat = work.tile([128, QT, KVH * HD], BF16, tag="v_nat", name="v_nat")
            for (wT, dst, nout) in ((wqT, q_nat, H * HD), (wkT, k_nat, KVH * HD),
                                    (wvT, v_nat, KVH * HD)):
                for nch in range(nout // 512):
                    pq0 = ps.tile([128, 512], F32, tag="ps512", name="pq0")
                    pq1 = ps.tile([128, 512], F32, tag="ps512", name="pq1")
                    for b in range(DB):
                        wtile = wload.tile([128, 512], BF16, tag="w_qkv", name="wtile")
                        nc.sync.dma_start(
                            wtile[:],
                            wT.rearrange("(b p) n -> p b n", p=128)[:, b, ts(nch, 512)],
                        )
                        nc.tensor.matmul(pq0[:], xn[:, b, ts(0, 128)], wtile[:],
                                         start=(b == 0), stop=(b == DB - 1))
                        nc.tensor.matmul(pq1[:], xn[:, b, ts(1, 128)], wtile[:],
                                         start=(b == 0), stop=(b == DB - 1))
                    nc.vector.tensor_copy(dst[:, 0, ts(nch, 512)], pq0[:])
                    nc.vector.tensor_copy(dst[:, 1, ts(nch, 512)], pq1[:])

            # ======== qk-norm + rope (in natural layout, in-place) ========
            def qknorm_rope(z, nh, w_sb):
                z3 = z[:].rearrange("p q (h d) -> p q h d", h=nh)
                HG = 2
                for qt in range(QT):
                    for hg in range(nh // HG):
                        zz = z3[:, qt, ts(hg, HG)]
                        sqz = work.tile([128, HG, HD], F32, tag="qk_sq", name="sqz")
                        nc.vector.tensor_tensor(sqz[:], zz, zz, ALU.mult)
                        ss = work.tile([128, HG, 1], F32, tag="qk_ss", name="ss")
                        nc.vector.reduce_sum(ss[:], sqz[:], axis=AX.X)
                        nc.vector.tensor_scalar(ss[:], ss[:], 1.0 / HD, EPS, ALU.mult, ALU.add)
                        sc_ = rsqrt_refined(ss[:], [128, HG, 1], f"qk{nh}")
                        nc.vector.tensor_tensor(zz, zz, sc_[:].to_broadcast([128, HG, HD]), ALU.mult)
                        nc.vector.tensor_tensor(
                            zz, zz, w_sb[:, None, :].to_broadcast([128, HG, HD]), ALU.mult
                        )
                        cosb = cos_sb[:, qt, None, :].to_broadcast([128, HG, HD // 2])
                        sinb = sin_sb[:, qt, None, :].to_broadcast([128, HG, HD // 2])
                        z1 = zz[:, :, : HD // 2]
                        z2 = zz[:, :, HD // 2 :]
                        a_ = work.tile([128, HG, HD // 2], F32, tag="rope_a", name="ra")
                        b_ = work.tile([128, HG, HD // 2], F32, tag="rope_b", name="rb")
                        c_ = work.tile([128, HG, HD // 2], F32, tag="rope_c", name="rc")
                        d_ = work.tile([128, HG, HD // 2], F32, tag="rope_d", name="rd")
                        nc.vector.tensor_tensor(a_[:], z1, cosb, ALU.mult)
                        nc.vector.tensor_tensor(b_[:], z2, sinb, ALU.mult)
                        nc.vector.tensor_tensor(c_[:], z2, cosb, ALU.mult)
                        nc.vector.tensor_tensor(d_[:], z1, sinb, ALU.mult)
                        nc.vector.tensor_tensor(z1, a_[:], b_[:], ALU.subtract)
                        nc.vector.tensor_tensor(z2, c_[:], d_[:], ALU.add)

            qknorm_rope(q_nat, H, qnw_sb)
            qknorm_rope(k_nat, KVH, knw_sb)

            if debug:
                nc.sync.dma_start(dbg_q.rearrange("(q p) n -> p q n", p=128), q_nat[:])
                nc.sync.dma_start(dbg_k.rearrange("(q p) n -> p q n", p=128), k_nat[:])

            # ======== KV AllGather ========
            for qt in range(QT):
                nc.sync.dma_start(ag_kv_in[ts(qt, 128), :512], k_nat[:, qt])
                nc.sync.dma_start(ag_kv_in[ts(qt, 128), 512:], v_nat[:, qt])
            nc.gpsimd.collective_compute(
                "AllGather", ALU.bypass, replica_groups=[list(range(NCORE))],
                ins=[ag_kv_in[:].opt()], outs=[ag_kv_out[:].opt()],
            )

            # K^T transposes + V bf16, k-blocks reordered [evens | odds]
            kT = big.tile([128, KVH, 16, 128], BF16, tag="K", name="kT")
            v_bf = big.tile([128, 16, KVH * HD], BF16, tag="V", name="v_bf")
            for j, cb in enumerate(KORD):
                kv_sb = work.tile([128, 1024], BF16, tag="kv_sb", name="kv_sb")
                nc.sync.dma_start(kv_sb[:], ag_kv_out[ts(cb, 128)])
                nc.vector.tensor_copy(v_bf[:, j], kv_sb[:, 512:])
                for hk in range(KVH):
                    ptp = ps.tile([128, 128], BF16, tag="pstp", name="ktp")
                    nc.tensor.transpose(ptp[:], kv_sb[:, ts(hk, 128)], ident_b[:])
                    nc.vector.tensor_copy(kT[:, hk, j], ptp[:])

            # Q^T transposes
            qT = big.tile([128, QT, H, 128], BF16, tag="Q", name="qT")
            for qt in range(QT):
                for h in range(H):
                    ptp = ps.tile([128, 128], BF16, tag="pstp", name="qtp")
                    nc.tensor.transpose(ptp[:], q_nat[:, qt, ts(h, 128)], ident_b[:])
                    nc.vector.tensor_copy(qT[:, qt, h], ptp[:])

            # ======== attention ========
            o_nat = big.tile([128, QT, H * HD], F32, tag="A", name="o_nat")
            for h in range(H):
                hk = h // (H // KVH)
                for qt in range(QT):
                    # chunks of 256 k-tokens over reordered blocks
                    if qt == 0:
                        chunks = [(2 * j, j) for j in range(4)]            # evens, masked
                    else:
                        chunks = [(2 * j, None) for j in range(4)] + \
                                 [(8 + 2 * j, j) for j in range(4)]        # evens free, odds masked
                    ncks = len(chunks)
                    den = work.tile([128, 8], F32, tag="den", name="den")
                    o_ps = ps.tile([128, 128], F32, tag="o_acc", name="o_ps")
                    for ci, (jb, mi) in enumerate(chunks):
                        s_ps = ps.tile([128, 256], F32, tag="ps512", name="s_ps")
                        nc.tensor.matmul(
                            s_ps[:], qT[:, qt, h],
                            kT[:, hk].rearrange("p b k -> p (b k)")[:, ds(jb * 128, 256)],
                            start=True, stop=True,
                        )
                        p_bf = work.tile([128, 256], BF16, tag="p_bf", name="p_bf")
                        if mi is None:
                            nc.scalar.activation(p_bf[:], s_ps[:], ACTF.Exp,
                                                 scale=SCALE, accum_out=den[:, ci : ci + 1])
                        else:
                            nc.scalar.activation(p_bf[:], s_ps[:], ACTF.Exp, scale=SCALE)
                            nc.vector.scalar_tensor_tensor(
                                p_bf[:], p_bf[:], 1.0, mask_sb[:, mi], ALU.mult, ALU.mult,
                                accum_out=den[:, ci : ci + 1],
                            )
                        for half in range(2):
                            pt_ps = ps.tile([128, 128], BF16, tag="pstp", name="pt_ps")
                            nc.tensor.transpose(pt_ps[:], p_bf[:, ts(half, 128)], ident_b[:])
                            pt_sb = work.tile([128, 128], BF16, tag="pt_sb", name="pt_sb")
                            nc.vector.tensor_copy(pt_sb[:], pt_ps[:])
                            nc.tensor.matmul(
                                o_ps[:], pt_sb[:], v_bf[:, jb + half, ts(hk, 128)],
                                start=(ci == 0 and half == 0),
                                stop=(ci == ncks - 1 and half == 1),
                            )
                    dsum = work.tile([128, 1], F32, tag="dsum", name="dsum")
                    nc.vector.reduce_sum(dsum[:], den[:, :ncks], axis=AX.X)
                    drec = work.tile([128, 1], F32, tag="drec", name="drec")
                    nc.vector.reciprocal(drec[:], dsum[:])
                    nc.vector.tensor_scalar(o_nat[:, qt, ts(h, 128)], o_ps[:], drec[:],
                                            None, ALU.mult)

            if debug:
                nc.sync.dma_start(dbg_o.rearrange("(q p) n -> p q n", p=128), o_nat[:])

            # ======== o^T + O-projection + residual -> h1^T ========
            oT = big.tile([128, H, QT, 128], BF16, tag="O", name="oT")
            for qt in range(QT):
                for hb in range(H):
                    ptp = ps.tile([128, 128], F32, tag="pstp", name="otp")
                    nc.tensor.transpose(ptp[:], o_nat[:, qt, ts(hb, 128)], ident_f[:])
                    nc.vector.tensor_copy(oT[:, hb, qt], ptp[:])
            h1T = big.tile([128, DB, TPC], F32, tag="H", name="h1T")
            for db in range(DB):
                wo_col = wload.tile([128, H, 128], BF16, tag="wo_col", name="wo_col")
                nc.sync.dma_start(
                    wo_col[:], woT.rearrange("(b p) n -> p b n", p=128)[:, :, ts(db, 128)]
                )
                ph = ps.tile([128, TPC], F32, tag="ps512", name="ph")
                for hb in range(H):
                    nc.tensor.matmul(
                        ph[:], wo_col[:, hb],
                        oT[:, hb].rearrange("p q t -> p (q t)"),
                        start=(hb == 0), stop=(hb == H - 1),
                    )
                nc.vector.tensor_tensor(h1T[:, db], ph[:], xw[:, db], ALU.add)

            if debug:
                nc.sync.dma_start(dbg_h1T.rearrange("(b p) t -> p b t", p=128), h1T[:])

            # ======== t = rms(h1)*ln2 (bf16), gate from h1T (fp32) ========
            ssq2 = ps_acc.tile([128, TPC], F32, tag="pd0", name="ssq2")
            for b in range(DB):
                sqb = work.tile([128, TPC], F32, tag="sqb", name="sqb2")
                nc.vector.tensor_tensor(sqb[:], h1T[:, b], h1T[:, b], ALU.mult)
                nc.tensor.matmul(ssq2[:], ones_f[:], sqb[:], start=(b == 0), stop=(b == DB - 1))
            m2 = work.tile([128, TPC], F32, tag="m_rms", name="m2")
            nc.vector.tensor_scalar(m2[:], ssq2[:], 1.0 / D, EPS, ALU.mult, ALU.add)
            s_t = rsqrt_refined(m2[:], [128, TPC], "t")
            tT_bf = big.tile([128, DB, TPC], BF16, tag="T", name="tT_bf")
            for b in range(DB):
                nc.vector.scalar_tensor_tensor(
                    tT_bf[:, b], h1T[:, b], ln2_sb[:, b : b + 1], s_t[:],
                    ALU.mult, ALU.mult,
                )

            # t natural (bf16) -> AllGather (dense-FFN source)
            t_nat = big.tile([128, QT, D], BF16, tag="V", name="t_nat")
            for qt in range(QT):
                for db in range(DB):
                    ptp = ps.tile([128, 128], BF16, tag="pstp", name="ttp")
                    nc.tensor.transpose(ptp[:], tT_bf[:, db, ts(qt, 128)], ident_b[:])
                    nc.vector.tensor_copy(t_nat[:, qt, ts(db, 128)], ptp[:])
            nc.sync.dma_start(ag_t_in.rearrange("(q p) d -> p q d", p=128), t_nat[:])
            nc.gpsimd.collective_compute(
                "AllGather", ALU.bypass, replica_groups=[list(range(NCORE))],
                ins=[ag_t_in[:].opt()], outs=[t_full[:].opt()],
            )
            if debug:
                nc.sync.dma_start(dbg_t.rearrange("(q p) d -> p q d", p=128), t_nat[:])

            # gate scores: z = (gate_w*ln2)^T h1 scaled per-token, sigmoid
            gw2 = cpool.tile([128, DB, E], F32)
            nc.vector.tensor_tensor(
                gw2[:], gwT_sb[:], ln2_sb[:, :, None].to_broadcast([128, DB, E]), ALU.mult
            )
            sc_nat = work.tile([128, QT, E], F32, tag="sc_nat", name="sc_nat")
            for qt in range(QT):
                pz = ps.tile([128, E], F32, tag="pstp", name="pz")
                for b in range(DB):
                    nc.tensor.matmul(pz[:], h1T[:, b, ts(qt, 128)], gw2[:, b],
                                     start=(b == 0), stop=(b == DB - 1))
                stp = ps.tile([128, 128], F32, tag="pstp", name="stp")
                nc.tensor.transpose(stp[:, :1], s_t[:1, ts(qt, 128)], ident_f[:1, :1])
                scol = work.tile([128, 1], F32, tag="scol", name="scol")
                nc.vector.tensor_copy(scol[:], stp[:, :1])
                nc.scalar.activation(sc_nat[:, qt], pz[:], ACTF.Sigmoid, scale=scol[:])
            nc.sync.dma_start(ag_s_in.rearrange("(q p) e -> p q e", p=128), sc_nat[:])
            nc.gpsimd.collective_compute(
                "AllGather", ALU.bypass, replica_groups=[list(range(NCORE))],
                ins=[ag_s_in[:].opt()], outs=[sc_full[:].opt()],
            )

            # ======== routing (replicated over all 2048 tokens) ========
            sc_all = route.tile([128, 16, E], F32, tag="sc_all", name="sc_all")
            nc.sync.dma_start(sc_all[:], sc_full.rearrange("(tb p) e -> p tb e", p=128))
            C_all = route.tile([128, 16, E], F32, tag="C_all", name="C_all")
            CeA = [route.tile([128, 16], F32, tag=f"CeA{k}", name=f"CeA{k}") for k in range(2)]
            for tb in range(16):
                sc_tb = sc_all[:, tb]
                s2 = route.tile([128, E], F32, tag="s2", name="s2")
                nc.vector.tensor_tensor(s2[:], sc_tb, gb_sb[:], ALU.add)
                gs = route.tile([128, G], F32, tag="gs", name="gs")
                grp_pad = route.tile([128, 8], F32, tag="grp_pad", name="grp_pad")
                for g in range(G):
                    nc.vector.memset(grp_pad[:], -1e30)
                    nc.vector.tensor_copy(grp_pad[:, :4], s2[:, ts(g, 4)])
                    mx8 = route.tile([128, 8], F32, tag="mx8", name="mx8")
                    mi8 = route.tile([128, 8], U32, tag="mi8", name="mi8")
                    nc.vector.max_with_indices(mx8[:], mi8[:], grp_pad[:])
                    nc.vector.tensor_tensor(gs[:, g : g + 1], mx8[:, 0:1], mx8[:, 1:2], ALU.add)
                gpad2 = route.tile([128, 8], F32, tag="gpad2", name="gpad2")
                nc.vector.memset(gpad2[:], -1e30)
                nc.vector.tensor_copy(gpad2[:, :4], gs[:])
                gv8 = route.tile([128, 8], F32, tag="gv8", name="gv8")
                gi8 = route.tile([128, 8], U32, tag="gi8", name="gi8")
                nc.vector.max_with_indices(gv8[:], gi8[:], gpad2[:])
                gmask = route.tile([128, G], F32, tag="gmask", name="gmask")
                nc.vector.tensor_scalar(gmask[:], gs[:], gv8[:, 1:2], None, ALU.is_ge)
                masked = route.tile([128, E], F32, tag="masked", name="masked")
                nc.vector.tensor_tensor(
                    masked[:].rearrange("p (g e) -> p g e", g=G),
                    s2[:].rearrange("p (g e) -> p g e", g=G),
                    gmask[:, :, None].to_broadcast([128, G, E // G]),
                    ALU.mult,
                )
                mv = route.tile([128, 8], F32, tag="mv", name="mv")
                mi = route.tile([128, 8], U32, tag="mi", name="mi")
                nc.vector.max_with_indices(mv[:], mi[:], masked[:])
                mi_f = route.tile([128, TOPK], F32, tag="mi_f", name="mi_f")
                nc.vector.tensor_copy(mi_f[:], mi[:, :TOPK])
                w4 = route.tile([128, TOPK], F32, tag="w4", name="w4")
                ohs = []
                for j in range(TOPK):
                    oh = route.tile([128, E], F32, tag=f"oh{j}", name=f"oh{j}")
                    nc.vector.tensor_scalar(oh[:], iota16[:], mi_f[:, j : j + 1], None,
                                            ALU.is_equal)
                    scratch = route.tile([128, E], F32, tag="scr", name="scr")
                    nc.vector.scalar_tensor_tensor(
                        scratch[:], oh[:], 1.0, sc_tb, ALU.mult, ALU.mult,
                        accum_out=w4[:, j : j + 1],
                    )
                    ohs.append(oh)
                wsum = route.tile([128, 1], F32, tag="wsum", name="wsum")
                nc.vector.reduce_sum(wsum[:], w4[:], axis=AX.X)
                nc.vector.tensor_scalar(wsum[:], wsum[:], 1e-20, None, ALU.add)
                winv = route.tile([128, 1], F32, tag="winv", name="winv")
                nc.vector.reciprocal(winv[:], wsum[:])
                wn4 = route.tile([128, TOPK], F32, tag="wn4", name="wn4")
                nc.vector.tensor_scalar(wn4[:], w4[:], winv[:], RSF, ALU.mult, ALU.mult)
                C_tb = C_all[:, tb]
                nc.vector.tensor_scalar(C_tb, ohs[0][:], wn4[:, 0:1], None, ALU.mult)
                for j in range(1, TOPK):
                    nc.vector.scalar_tensor_tensor(
                        C_tb, ohs[j][:], wn4[:, j : j + 1], C_tb, ALU.mult, ALU.add
                    )
                for k in range(2):
                    scr2 = route.tile([128, E], F32, tag="scr2", name="scr2")
                    nc.vector.scalar_tensor_tensor(
                        scr2[:], C_tb, 1.0, esel_sb[k][:, :E], ALU.mult, ALU.mult,
                        accum_out=CeA[k][:, tb : tb + 1],
                    )

            if debug:
                nc.sync.dma_start(dbg_C.rearrange("(tb p) e -> p tb e", p=128), C_all[:])

            # ======== dense expert FFN (2 experts over all tokens) ========
            # process tokens in 2 chunks of 1024; everything plain-DMA
            TCH = 512
            for tch in range(L // TCH):
                # t_full^T for this token chunk (PE transposes of the AG output)
                tfT = big.tile([128, DB, TCH], BF16, tag="K", name=f"tfT{tch}")
                for pb in range(TCH // 128):
                    trow = work.tile([128, D], BF16, tag="kv_sb", name="trow")
                    nc.sync.dma_start(trow[:], t_full[ts(tch * (TCH // 128) + pb, 128)])
                    for db in range(DB):
                        ptp = ps.tile([128, 128], BF16, tag="pstp", name="ttp2")
                        nc.tensor.transpose(ptp[:], trow[:, ts(db, 128)], ident_b[:])
                        nc.vector.tensor_copy(tfT[:, db, ts(pb, 128)], ptp[:])
                hm2 = []
                for k in range(2):
                    hm = big.tile([128, I_FF // 128, TCH], BF16, tag=("O" if k == 0 else "Q"),
                                  name=f"hm{k}_{tch}")
                    for mat_i, mat in enumerate((wg_p, wu_p)):
                        for ich in range(I_FF // 128):
                            wmat = wload.tile([128, DB, 128], BF16, tag="w_exp", name="wmat")
                            nc.sync.dma_start(
                                wmat[:],
                                mat.rearrange("e (b p) i -> p e b i", p=128)[:, k, :, ts(ich, 128)],
                            )
                            for nch in range(TCH // 512):
                                pg = ps.tile([128, 512], F32, tag="ps512", name="pg")
                                for b in range(DB):
                                    nc.tensor.matmul(
                                        pg[:], wmat[:, b], tfT[:, b, ts(nch, 512)],
                                        start=(b == 0), stop=(b == DB - 1),
                                    )
                                if mat_i == 0:
                                    nc.scalar.activation(hm[:, ich, ts(nch, 512)],
                                                         pg[:], ACTF.Silu)
                                else:
                                    nc.vector.tensor_tensor(
                                        hm[:, ich, ts(nch, 512)],
                                        hm[:, ich, ts(nch, 512)], pg[:], ALU.mult,
                                    )
                    hm2.append(hm)
                # down projection for both experts, combine with per-token weights
                for dch in range(D // 256):
                    wd0 = wload.tile([128, I_FF // 128, 256], BF16, tag="wd0", name="wd0")
                    nc.sync.dma_start(
                        wd0[:], wd_p.rearrange("e (b p) d -> p e b d", p=128)[:, 0, :, ts(dch, 256)]
                    )
                    wd1 = wload.tile([128, I_FF // 128, 256], BF16, tag="wd1", name="wd1")
                    nc.sync.dma_start(
                        wd1[:], wd_p.rearrange("e (b p) d -> p e b d", p=128)[:, 1, :, ts(dch, 256)]
                    )
                    for tb in range(TCH // 128):
                        tbg = tch * (TCH // 128) + tb
                        py0 = ps_acc.tile([128, 256], F32, tag="pd0", name="py0")
                        for ib in range(I_FF // 128):
                            nc.tensor.matmul(py0[:], hm2[0][:, ib, ts(tb, 128)], wd0[:, ib],
                                             start=(ib == 0), stop=(ib == I_FF // 128 - 1))
                        py1 = ps_acc.tile([128, 256], F32, tag="pd1", name="py1")
                        for ib in range(I_FF // 128):
                            nc.tensor.matmul(py1[:], hm2[1][:, ib, ts(tb, 128)], wd1[:, ib],
                                             start=(ib == 0), stop=(ib == I_FF // 128 - 1))
                        ycmb = work.tile([128, 256], F32, tag="ycmb", name="ycmb")
                        nc.vector.tensor_scalar(ycmb[:], py0[:], CeA[0][:, tbg : tbg + 1],
                                                None, ALU.mult)
                        nc.vector.scalar_tensor_tensor(
                            ycmb[:], py1[:], CeA[1][:, tbg : tbg + 1], ycmb[:],
                            ALU.mult, ALU.add,
                        )
                        ybf = work.tile([128, 256], BF16, tag="p_bf", name="ybf")
                        nc.vector.tensor_copy(ybf[:], ycmb[:])
                        nc.sync.dma_start(y_dram[ts(tbg, 128), ts(dch, 256)], ybf[:])

            # ======== ReduceScatter y ========
            nc.gpsimd.collective_compute(
                "ReduceScatter", ALU.add, replica_groups=[list(range(NCORE))],
                ins=[y_dram[:].opt()], outs=[y_shard[:].opt()],
            )

            # ======== shared expert ========
            hms = big.tile([128, SH_I // 128, TPC], BF16, tag="Q", name="hms")
            for mat_i, mat in enumerate((shgT, shuT)):
                for shc in range(SH_I // 128):
                    wcol_sh = wload.tile([128, DB, 128], BF16, tag="w_exp", name="wcol_sh")
                    nc.sync.dma_start(
                        wcol_sh[:], mat.rearrange("(b p) n -> p b n", p=128)[:, :, ts(shc, 128)]
                    )
                    pgs = ps.tile([128, TPC], F32, tag="ps512", name="pgs")
                    for b in range(DB):
                        nc.tensor.matmul(pgs[:], wcol_sh[:, b], tT_bf[:, b],
                                         start=(b == 0), stop=(b == DB - 1))
                    if mat_i == 0:
                        nc.scalar.activation(hms[:, shc], pgs[:], ACTF.Silu)
                    else:
                        nc.vector.tensor_tensor(hms[:, shc], hms[:, shc], pgs[:], ALU.mult)

            # ======== final: out = h1 + y + sh (natural layout) ========
            y_sh_sb = big.tile([128, QT, D], BF16, tag="C", name="y_sh_sb")
            nc.sync.dma_start(y_sh_sb[:], y_shard.rearrange("(q p) d -> p q d", p=128))
            if debug:
                nc.sync.dma_start(dbg_y.rearrange("(q p) d -> p q d", p=128), y_sh_sb[:])
            h1n = big.tile([128, QT, D], BF16, tag="A", name="h1n")
            for qt in range(QT):
                for db in range(DB):
                    ptp = ps.tile([128, 128], F32, tag="pstp", name="h1tp")
                    nc.tensor.transpose(ptp[:], h1T[:, db, ts(qt, 128)], ident_f[:])
                    nc.vector.tensor_copy(h1n[:, qt, ts(db, 128)], ptp[:])
            for dch in range(D // 128):
                wdcol = wload.tile([128, SH_I // 128, 128], BF16, tag="w_exp", name="wdcol")
                nc.sync.dma_start(
                    wdcol[:], shdT.rearrange("(b p) d -> p b d", p=128)[:, :, ts(dch, 128)]
                )
                for qt in range(QT):
                    psh = ps.tile([128, 128], F32, tag="pstp", name="psh")
                    for sb_ in range(SH_I // 128):
                        nc.tensor.matmul(psh[:], hms[:, sb_, ts(qt, 128)], wdcol[:, sb_],
                                         start=(sb_ == 0), stop=(sb_ == SH_I // 128 - 1))
                    oc = work.tile([128, 128], F32, tag="ycmb", name="oc")
                    nc.vector.tensor_tensor(oc[:], psh[:], h1n[:, qt, ts(dch, 128)], ALU.add)
                    nc.vector.tensor_tensor(oc[:], oc[:], y_sh_sb[:, qt, ts(dch, 128)], ALU.add)
                    nc.sync.dma_start(
                        out_sh.rearrange("(q p) d -> p q d", p=128)[:, qt, ts(dch, 128)],
                        oc[:],
                    )

    return nc


# =====================================================================
# host side
# =====================================================================

_BUILD_CACHE = {}


def _get_nc(debug=False):
    if debug not in _BUILD_CACHE:
        nc = build(debug=debug)
        _fixup_multi_waits(nc)
        _BUILD_CACHE[debug] = nc
    return _BUILD_CACHE[debug]


def _prep_inputs(inputs):
    bf = ml_dtypes.bfloat16
    x = np.asarray(inputs["x"], np.float32).reshape(L, D)
    x_perm = x[PERM]
    wq, wk, wv, wo = (np.asarray(inputs[k], np.float32) for k in ("w_q", "w_k", "w_v", "w_o"))
    gate_w = np.asarray(inputs["gate_w"], np.float32)
    gate_bias = np.asarray(inputs["gate_bias"], np.float32)
    wg, wu, wd = (np.asarray(inputs[k], np.float32) for k in ("wg", "wu", "wd"))
    shg, shu, shd = (np.asarray(inputs[k], np.float32) for k in ("sh_g", "sh_u", "sh_d"))

    pos = np.arange(L, dtype=np.float32)
    inv = 1.0 / (THETA ** (np.arange(0, HD, 2, dtype=np.float32) / HD))
    ang = pos[:, None] * inv[None, :]
    cos_t, sin_t = np.cos(ang).astype(np.float32), np.sin(ang).astype(np.float32)

    common = {
        "wqT": np.ascontiguousarray(wq.T).astype(bf),
        "wkT": np.ascontiguousarray(wk.T).astype(bf),
        "wvT": np.ascontiguousarray(wv.T).astype(bf),
        "woT": np.ascontiguousarray(wo.T).astype(bf),
        "gwT": np.ascontiguousarray(gate_w.T),
        "gbias": gate_bias.reshape(1, E).astype(np.float32),
        "ln1pd": np.asarray(inputs["ln1_w"], np.float32).reshape(DB, 128).T.copy(),
        "ln2pd": np.asarray(inputs["ln2_w"], np.float32).reshape(DB, 128).T.copy(),
        "qnw": np.asarray(inputs["q_norm_w"], np.float32).reshape(1, HD),
        "knw": np.asarray(inputs["k_norm_w"], np.float32).reshape(1, HD),
        "shgT": np.ascontiguousarray(shg.T).astype(bf),
        "shuT": np.ascontiguousarray(shu.T).astype(bf),
        "shdT": np.ascontiguousarray(shd.T).astype(bf),
    }

    in_maps = []
    tri = np.tril(np.ones((128, 128), np.float32), -1)
    for c in range(NCORE):
        sl = slice(c * TPC, (c + 1) * TPC)
        abs_tok = PERM[sl]
        e0, e1 = PAIRS[c]
        es0 = np.zeros((1, CPAD), np.float32)
        es0[0, e0] = 1.0
        es1 = np.zeros((1, CPAD), np.float32)
        es1[0, e1] = 1.0
        # allowed(i, p, j) = j < p or (j == p and i <= c); pairs (2u, 2u+1)
        m = np.zeros((8, 128, 128), np.float32)
        for i in range(8):
            m[i] = tri + np.eye(128, dtype=np.float32) * (1.0 if i <= c else 0.0)
        mp = np.concatenate([m[::2], m[1::2]], axis=0)  # no-op order helper
        maskpair = np.zeros((4, 128, 256), np.float32)
        for u in range(4):
            maskpair[u, :, :128] = m[2 * u]
            maskpair[u, :, 128:] = m[2 * u + 1]
        im = {
            **common,
            "xT32": np.ascontiguousarray(x_perm[sl].T),
            "cosq": cos_t[abs_tok],
            "sinq": sin_t[abs_tok],
            "maskp": maskpair.astype(bf),
            "esel0": es0,
            "esel1": es1,
            "wg_p": np.ascontiguousarray(wg[[e0, e1]].transpose(0, 2, 1)).astype(bf),
            "wu_p": np.ascontiguousarray(wu[[e0, e1]].transpose(0, 2, 1)).astype(bf),
            "wd_p": np.ascontiguousarray(wd[[e0, e1]].transpose(0, 2, 1)).astype(bf),
        }
        in_maps.append(im)
    return in_maps


def run(inputs, debug=False):
    nc = _get_nc(debug=debug)
    in_maps = _prep_inputs(inputs)
    return run_bass_kernel_spmd(nc, in_maps, core_ids=list(range(NCORE)))


def kernel(**inputs) -> np.ndarray:
    res = run(inputs, debug=False)
    out_perm = np.concatenate([res.results[c]["out_sh"] for c in range(NCORE)], axis=0)
    out = np.empty((L, D), np.float32)
    out[PERM] = out_perm
    return out.reshape(1, L, D)
